# revision 2
# baseline (speedup 1.0000x reference)
"""3-layer GCN (GCNConv x3 + FC) on 8 Trainium2 NeuronCores.

Strategy (graph/data parallel, node sharding):
  - Nodes partitioned contiguously: core c owns dst rows [c*6250, (c+1)*6250).
  - Propagate commutes with the dense weight matmul (A(HW) == (AH)W), so
    message passing always runs at width 128/128/32 instead of 256.
  - Per-edge coefficient norm_e = dis[src]*dis[dst] (dis = 1/sqrt(deg+1))
    is a pure function of edge_index; precomputed on host like standard
    GNN preprocessing (PyG caches the same per-edge norm).
  - Self-loops are materialized as ordinary edges with norm = dis^2.
  - On device, per 128-edge k-tile: indirect-DMA gather of the 128 source
    rows, DVE builds S[e,m] = (m == dstslot[e]) * norm[e] in one
    tensor_scalar op, PE accumulates psum[F, slots] += msg.T @ S.
  - Dense stages run feature-major (transpose-free); biases become
    per-partition ACT bias, leaky-relu is ACT Lrelu.
  - Cross-layer halo exchange degenerates to AllGather of the next
    propagate input (width 128 after layer 1, width 32 after layer 2).
"""

import sys

if "/opt/trn_rl_repo" not in sys.path:
    sys.path.insert(0, "/opt/trn_rl_repo")

import numpy as np

import concourse.bass as bass
import concourse.tile as tile
import concourse.mybir as mybir
from concourse import bacc
from concourse.bass_utils import run_bass_kernel_spmd

N = 50000
E = 800000
NCORES = 8
SH = N // NCORES          # 6250 nodes per core
P = 128
NT = (SH + P - 1) // P    # 49 dst tiles per core (48 full + 1 of 106)
LAST = SH - (NT - 1) * P  # 106
NEG_SLOPE = 0.01
PAD_SLOT = 300.0

F32 = mybir.dt.float32
I32 = mybir.dt.int32
AF = mybir.ActivationFunctionType


def _build_tables(edge_index):
    """Sort/pad edges into per-core [P, KT] gather tables.

    Returns (nk, src_tab, slot_tab, norm_tab) where nk[t] is the (uniform
    across cores) number of 128-edge k-tiles for dst tile t, and the tabs
    are [NCORES, P, KT] with column j holding k-tile j's 128 edges.
    """
    src = edge_index[0].astype(np.int64)
    dst = edge_index[1].astype(np.int64)
    loop = np.arange(N, dtype=np.int64)
    s = np.concatenate([src, loop])
    d = np.concatenate([dst, loop])
    deg = np.bincount(d, minlength=N)
    dis = 1.0 / np.sqrt(deg.astype(np.float64))
    norm = (dis[s] * dis[d]).astype(np.float32)

    core = d // SH
    tloc = (d % SH) // P
    gt = core * NT + tloc
    order = np.lexsort((s, gt))
    s, norm, gt = s[order], norm[order], gt[order]
    slot = ((d[order] % SH) % P).astype(np.float32)

    counts = np.bincount(gt, minlength=NCORES * NT).reshape(NCORES, NT)
    nk = np.maximum(np.max((counts + P - 1) // P, axis=0), 1).astype(np.int64)
    KT = int(nk.sum())
    col_off = np.concatenate([[0], np.cumsum(nk)]).astype(np.int64)

    src_tab = np.zeros((NCORES, KT, P), np.int32)
    slot_tab = np.full((NCORES, KT, P), PAD_SLOT, np.float32)
    norm_tab = np.zeros((NCORES, KT, P), np.float32)
    starts = np.concatenate([[0], np.cumsum(counts.reshape(-1))]).astype(np.int64)
    for c in range(NCORES):
        for t in range(NT):
            g = c * NT + t
            a, b = starts[g], starts[g + 1]
            cnt = b - a
            j0 = col_off[t]
            blk = src_tab[c][j0 : j0 + nk[t]].reshape(-1)
            blk[:cnt] = s[a:b]
            blk = slot_tab[c][j0 : j0 + nk[t]].reshape(-1)
            blk[:cnt] = slot[a:b]
            blk = norm_tab[c][j0 : j0 + nk[t]].reshape(-1)
            blk[:cnt] = norm[a:b]
    src_tab = np.ascontiguousarray(src_tab.transpose(0, 2, 1))
    slot_tab = np.ascontiguousarray(slot_tab.transpose(0, 2, 1))
    norm_tab = np.ascontiguousarray(norm_tab.transpose(0, 2, 1))
    return nk, src_tab, slot_tab, norm_tab


def _build_program(nk):
    KT = int(nk.sum())
    nc = bacc.Bacc("TRN2", target_bir_lowering=False, debug=False,
                   num_devices=NCORES)

    x_t = nc.dram_tensor("x", [N, 128], F32, kind="ExternalInput")
    w1_t = nc.dram_tensor("W1", [128, 256], F32, kind="ExternalInput")
    b1_t = nc.dram_tensor("b1", [256], F32, kind="ExternalInput")
    w2_t = nc.dram_tensor("W2", [256, 128], F32, kind="ExternalInput")
    b2_t = nc.dram_tensor("b2", [128], F32, kind="ExternalInput")
    w3_t = nc.dram_tensor("W3", [128, 32], F32, kind="ExternalInput")
    b3_t = nc.dram_tensor("b3", [32], F32, kind="ExternalInput")
    wfc_t = nc.dram_tensor("Wfc", [32, 1], F32, kind="ExternalInput")
    bfc_t = nc.dram_tensor("bfc", [1], F32, kind="ExternalInput")
    src_t = nc.dram_tensor("src", [P, KT], I32, kind="ExternalInput")
    slot_t = nc.dram_tensor("slot", [P, KT], F32, kind="ExternalInput")
    norm_t = nc.dram_tensor("norm", [P, KT], F32, kind="ExternalInput")
    y_t = nc.dram_tensor("y", [SH], F32, kind="ExternalOutput")

    with tile.TileContext(nc) as tc:
        with tc.tile_pool(name="const", bufs=1) as cpool, \
             tc.tile_pool(name="tabs", bufs=1) as tabpool, \
             tc.tile_pool(name="gather", bufs=8) as gpool, \
             tc.tile_pool(name="sel", bufs=8) as spool, \
             tc.tile_pool(name="dense", bufs=3) as dpool, \
             tc.tile_pool(name="acc", bufs=2, space="PSUM") as acc_pool, \
             tc.tile_pool(name="dpsum", bufs=2, space="PSUM") as dps_pool, \
             tc.tile_pool(name="dram", bufs=1, space="DRAM") as drampool:

            # --- constants ---
            iota_i = cpool.tile([P, P], I32)
            nc.gpsimd.iota(iota_i[:], pattern=[[1, P]], base=0,
                           channel_multiplier=0)
            iota_f = cpool.tile([P, P], F32)
            nc.vector.tensor_copy(out=iota_f[:], in_=iota_i[:])

            w1 = cpool.tile([128, 256], F32)
            nc.sync.dma_start(out=w1[:], in_=w1_t[:])
            w2a = cpool.tile([128, 128], F32)
            nc.sync.dma_start(out=w2a[:], in_=w2_t[0:128, :])
            w2b = cpool.tile([128, 128], F32)
            nc.sync.dma_start(out=w2b[:], in_=w2_t[128:256, :])
            w3 = cpool.tile([128, 32], F32)
            nc.sync.dma_start(out=w3[:], in_=w3_t[:])
            wfc = cpool.tile([32, 1], F32)
            nc.sync.dma_start(out=wfc[:], in_=wfc_t[:])
            b1a = cpool.tile([128, 1], F32)
            nc.sync.dma_start(out=b1a[:], in_=b1_t[0:128, None])
            b1b = cpool.tile([128, 1], F32)
            nc.sync.dma_start(out=b1b[:], in_=b1_t[128:256, None])
            b2 = cpool.tile([128, 1], F32)
            nc.sync.dma_start(out=b2[:], in_=b2_t[:, None])
            b3 = cpool.tile([32, 1], F32)
            nc.sync.dma_start(out=b3[:], in_=b3_t[:, None])
            bfc = cpool.tile([1, 1], F32)
            nc.sync.dma_start(out=bfc[:], in_=bfc_t[:, None])

            y_row = cpool.tile([1, SH], F32)

            # --- intermediate node tables ---
            q2_shard = drampool.tile([SH, 128], F32)
            q2_full = drampool.tile([N, 128], F32, addr_space="Shared")
            q3_shard = drampool.tile([SH, 32], F32)
            q3_full = drampool.tile([N, 32], F32, addr_space="Shared")

            col_off = np.concatenate([[0], np.cumsum(nk)]).astype(int)

            src_s = tabpool.tile([P, KT], I32, tag="src")
            nc.sync.dma_start(out=src_s[:], in_=src_t[:])
            slot_s = tabpool.tile([P, KT], F32, tag="slot")
            nc.sync.dma_start(out=slot_s[:], in_=slot_t[:])
            norm_s = tabpool.tile([P, KT], F32, tag="norm")
            nc.sync.dma_start(out=norm_s[:], in_=norm_t[:])
            tabs = (src_s, slot_s, norm_s)

            def propagate_tile(t, table_ap, F, tabs):
                """Returns PSUM tile [F, P] = sum of norm-scaled gathered rows."""
                src_s, slot_s, norm_s = tabs
                acc = acc_pool.tile([F, P], F32, tag="acc", space="PSUM")
                n_k = int(nk[t])
                j0 = int(col_off[t])
                for k in range(n_k):
                    j = j0 + k
                    msg = gpool.tile([P, F], F32, tag="msg")
                    nc.gpsimd.indirect_dma_start(
                        out=msg[:], out_offset=None, in_=table_ap,
                        in_offset=bass.IndirectOffsetOnAxis(
                            ap=src_s[:, j : j + 1], axis=0))
                    S = spool.tile([P, P], F32, tag="S")
                    nc.vector.tensor_scalar(
                        out=S[:], in0=iota_f[:],
                        scalar1=slot_s[:, j : j + 1],
                        scalar2=norm_s[:, j : j + 1],
                        op0=mybir.AluOpType.is_equal,
                        op1=mybir.AluOpType.mult)
                    nc.tensor.matmul(out=acc[:], lhsT=msg[:], rhs=S[:],
                                     start=(k == 0), stop=(k == n_k - 1))
                return acc

            # ---------------- layer 1 ----------------
            for t in range(NT):
                nv = P if t < NT - 1 else LAST
                r0 = t * P
                acc = propagate_tile(t, x_t[:], 128, tabs)
                p1 = dpool.tile([128, P], F32, tag="p1")
                nc.vector.tensor_copy(out=p1[:], in_=acc[:])
                h1a_ps = dps_pool.tile([128, P], F32, tag="da", space="PSUM")
                nc.tensor.matmul(out=h1a_ps[:], lhsT=w1[:, 0:128], rhs=p1[:],
                                 start=True, stop=True)
                h1b_ps = dps_pool.tile([128, P], F32, tag="db", space="PSUM")
                nc.tensor.matmul(out=h1b_ps[:], lhsT=w1[:, 128:256], rhs=p1[:],
                                 start=True, stop=True)
                h1a = dpool.tile([128, P], F32, tag="h1a")
                nc.scalar.activation(out=h1a[:], in_=h1a_ps[:], func=AF.Lrelu,
                                     bias=b1a[:, :1], scale=1.0, alpha=NEG_SLOPE)
                h1b = dpool.tile([128, P], F32, tag="h1b")
                nc.scalar.activation(out=h1b[:], in_=h1b_ps[:], func=AF.Lrelu,
                                     bias=b1b[:, :1], scale=1.0, alpha=NEG_SLOPE)
                q2_ps = dps_pool.tile([P, 128], F32, tag="dc", space="PSUM")
                nc.tensor.matmul(out=q2_ps[:], lhsT=h1a[:], rhs=w2a[:],
                                 start=True, stop=False)
                nc.tensor.matmul(out=q2_ps[:], lhsT=h1b[:], rhs=w2b[:],
                                 start=False, stop=True)
                q2_s = dpool.tile([P, 128], F32, tag="q2s")
                nc.scalar.activation(out=q2_s[:], in_=q2_ps[:], func=AF.Copy)
                nc.sync.dma_start(out=q2_shard[r0 : r0 + nv, :],
                                  in_=q2_s[:nv, :])

            nc.gpsimd.collective_compute(
                "AllGather", mybir.AluOpType.bypass,
                replica_groups=[list(range(NCORES))],
                ins=[q2_shard[:].opt()], outs=[q2_full[:].opt()])

            # ---------------- layer 2 ----------------
            for t in range(NT):
                nv = P if t < NT - 1 else LAST
                r0 = t * P
                acc = propagate_tile(t, q2_full[:], 128, tabs)
                h2 = dpool.tile([128, P], F32, tag="h2")
                nc.scalar.activation(out=h2[:], in_=acc[:], func=AF.Lrelu,
                                     bias=b2[:, :1], scale=1.0, alpha=NEG_SLOPE)
                q3_ps = dps_pool.tile([P, 32], F32, tag="dc", space="PSUM")
                nc.tensor.matmul(out=q3_ps[:], lhsT=h2[:], rhs=w3[:],
                                 start=True, stop=True)
                q3_s = dpool.tile([P, 32], F32, tag="q3s")
                nc.scalar.activation(out=q3_s[:], in_=q3_ps[:], func=AF.Copy)
                nc.sync.dma_start(out=q3_shard[r0 : r0 + nv, :],
                                  in_=q3_s[:nv, :])

            nc.gpsimd.collective_compute(
                "AllGather", mybir.AluOpType.bypass,
                replica_groups=[list(range(NCORES))],
                ins=[q3_shard[:].opt()], outs=[q3_full[:].opt()])

            # ---------------- layer 3 + FC ----------------
            for t in range(NT):
                nv = P if t < NT - 1 else LAST
                r0 = t * P
                acc = propagate_tile(t, q3_full[:], 32, tabs)
                h3 = dpool.tile([32, P], F32, tag="h3")
                nc.scalar.activation(out=h3[:], in_=acc[:], func=AF.Lrelu,
                                     bias=b3[:, :1], scale=1.0, alpha=NEG_SLOPE)
                o_ps = dps_pool.tile([1, P], F32, tag="dc", space="PSUM")
                nc.tensor.matmul(out=o_ps[:], lhsT=wfc[:], rhs=h3[:],
                                 start=True, stop=True)
                nc.scalar.activation(out=y_row[:1, r0 : r0 + nv],
                                     in_=o_ps[:1, :nv], func=AF.Identity,
                                     bias=bfc[:1, :1], scale=1.0)

            nc.sync.dma_start(out=y_t[None, :], in_=y_row[:1, :])

    nc.compile()
    return nc


def kernel(x, edge_index, W1, b1, W2, b2, W3, b3, Wfc, bfc, _trace=False):
    x = np.ascontiguousarray(np.asarray(x, np.float32))
    nk, src_tab, slot_tab, norm_tab = _build_tables(np.asarray(edge_index))
    nc = _build_program(nk)

    common = {
        "x": x,
        "W1": np.asarray(W1, np.float32), "b1": np.asarray(b1, np.float32),
        "W2": np.asarray(W2, np.float32), "b2": np.asarray(b2, np.float32),
        "W3": np.asarray(W3, np.float32), "b3": np.asarray(b3, np.float32),
        "Wfc": np.asarray(Wfc, np.float32), "bfc": np.asarray(bfc, np.float32),
    }
    in_maps = []
    for c in range(NCORES):
        m = dict(common)
        m["src"] = src_tab[c]
        m["slot"] = slot_tab[c]
        m["norm"] = norm_tab[c]
        in_maps.append(m)

    res = run_bass_kernel_spmd(nc, in_maps, core_ids=list(range(NCORES)),
                               trace=_trace)
    out = np.concatenate([res.results[c]["y"] for c in range(NCORES)])
    if _trace:
        kernel.last_results = res
    return out.astype(np.float32)
